# revision 4
# baseline (speedup 1.0000x reference)
"""Custom multi-head attention with stoichiometric bias on 8 Trainium2 cores.

Reference op (per batch b, head h, tokens i,j; T=4096, d_model=512, H=8, hd=64):
    Q = q @ Wq.T + bq ; K,V likewise (zero biases asserted).
    S = (Q_h K_h^T) / sqrt(hd)
    D[i,j] = frac[j] - frac[i]
    bias = ap_h * max(D,0) + an_h * min(D,0)
    out = softmax(S + bias) @ V_h  -> concat heads -> @ Wo.T + bo

Sharding: 16 (batch, head-pair) units -> core c handles batch b=c//4 and heads
(2*(c%4), 2*(c%4)+1); out_proj is row-parallel, host sums 4 partials per batch.

Key structure (v2):
  * The ENTIRE bias g(f_j - f_i) = an*D + c*relu(D) is folded into the QK^T
    matmul as 64 extra contraction rows: host-side SVD of the continuous
    kernel g(x-y) on [0,1]^2 gives rank-64 factors psi_r(f_j) (K side) and
    phi_r(f_i) (Q side); max bias error ~1e-3 logits.  The S matmul had 62
    idle contraction rows (64 of 128 used), so this is FREE on the PE, and it
    removes ALL per-element DVE work (the old exp(bias) tensor_scalar + mult):
    exp(psum) feeds the PV matmul directly.  Act engine (exp, 924ns per
    [128,1024] tile) is the sole pacer of the attention inner loop.
  * Scores are computed transposed, S^T[j,i]: softmax row sums come from a
    ones-column in V via the PV matmul, P^T feeds PV directly, no transposes.
  * No max-subtraction: logits are bounded (~6.5) for this op's distribution;
    bf16 holds e^s exactly fine.
  * Head-packed projections: K/Q proj matmuls use M=128 (both heads per
    stream).  Cross-partition copies are avoided by a per-head row layout:
    h0 tiles hold proj dims at partitions 0:64 and aug rows at 64:128,
    h1 tiles hold aug rows at 0:64 and proj dims at 64:128, so psum slices
    copy partition-aligned into both.
  * DMA queues: output tiles go out on the SP (sync) queue; the per-block
    softmax-sum row hop (lt) goes on the Act (scalar) queue so it never
    queues behind output DMAs (that interaction cost ~47us in v1); v-stage
    loads go on the gpsimd SWDGE queue so they don't queue behind k/q stages.
"""

import sys

import numpy as np
import ml_dtypes

for _p in ("/opt/trn_rl_repo", "/root/.axon_site/_ro/trn_rl_repo"):
    if _p not in sys.path:
        sys.path.append(_p)

import concourse.bass as bass  # noqa: E402
import concourse.mybir as mybir  # noqa: E402
import concourse.tile as tile  # noqa: E402
from concourse import bacc  # noqa: E402
from concourse.bass_utils import run_bass_kernel_spmd  # noqa: E402

BF16 = ml_dtypes.bfloat16
T = 4096
DM = 512
HD = 64
NHEAD = 8
N_CORES = 8
SCALE = HD ** -0.5
IB = 1024           # i-block width (free dim of score psum)
N_IB = T // IB      # 4
JC = 128            # j chunk (partition dim of S^T tiles)
N_JC = T // JC      # 32
KC = 128            # contraction chunk for projections
N_KC = DM // KC     # 4
RANK = 64           # bias factorization rank (aug contraction rows)
AUG_GRID = 1024     # host-side SVD grid for the bias kernel

f32 = mybir.dt.float32
bf16 = mybir.dt.bfloat16

_PROGRAM = None
PHASES = ("proj", "attn", "oproj")  # debug: subset of phases to build
REPS = 1
FAKE_LT = False  # timing diagnostic only: replace lt DMA with a memset


def _build_program(loop_reps=None):
    """Trace + compile the (input-independent) per-core Bass program.

    loop_reps: if set, wrap the body in a hardware For_i loop with that trip
    count (used by test.py to amortize fixed dispatch overhead when timing;
    kernel() itself always builds the single-shot program).
    """
    nc = bacc.Bacc("TRN2", target_bir_lowering=False)

    # DRAM I/O (per-core shapes)
    qT = nc.dram_tensor("qT", [DM, T], bf16, kind="ExternalInput")
    kT = nc.dram_tensor("kT", [DM, T], bf16, kind="ExternalInput")
    vT = nc.dram_tensor("vT", [DM, T], bf16, kind="ExternalInput")
    wq = nc.dram_tensor("wq", [DM, 128], bf16, kind="ExternalInput")
    wk = nc.dram_tensor("wk", [DM, 128], bf16, kind="ExternalInput")
    wv = nc.dram_tensor("wv", [DM, 128], bf16, kind="ExternalInput")
    wo = nc.dram_tensor("wo", [128, DM], bf16, kind="ExternalInput")
    kaug = nc.dram_tensor("kaug", [2, RANK, T], bf16, kind="ExternalInput")
    qaug = nc.dram_tensor("qaug", [2, RANK, T], bf16, kind="ExternalInput")
    out = nc.dram_tensor("out", [T, DM], f32, kind="ExternalOutput")

    with tile.TileContext(nc) as tc:
        with (
            tc.tile_pool(name="singles", bufs=1) as singles,
            tc.tile_pool(name="stage", bufs=8) as stage,
            tc.tile_pool(name="es", bufs=4) as esp,
            tc.tile_pool(name="outs", bufs=2) as outs,
            tc.tile_pool(name="rbp", bufs=4) as rbp,
            tc.tile_pool(name="pvsp", bufs=6) as pvsp,
            tc.tile_pool(name="spsum", bufs=2, space="PSUM") as spsum,
            tc.tile_pool(name="vpsum", bufs=1, space="PSUM") as vpsum,
            tc.tile_pool(name="opsum", bufs=2, space="PSUM") as opsum,
        ):
            # ---- persistent tiles ----
            # Row layout: h0 = [dims 0:64 | aug 64:128], h1 = [aug 0:64 | dims 64:128]
            QT = [singles.tile([128, T], bf16, name=f"qt{h}", tag=f"qt{h}") for h in range(2)]
            KT = [singles.tile([128, T], bf16, name=f"kt{h}", tag=f"kt{h}") for h in range(2)]
            VA = [singles.tile([128, 65 * N_JC], bf16, name=f"va{h}", tag=f"va{h}") for h in range(2)]
            WA = singles.tile([128, 3 * N_KC, 128], bf16, name="was", tag="was")
            WQ = WA[:, 0:N_KC]
            WK = WA[:, N_KC : 2 * N_KC]
            WV = WA[:, 2 * N_KC : 3 * N_KC]
            WO2 = singles.tile([64, 2 * DM], bf16, name="wos", tag="wos")
            WO = [WO2[:, 0:DM], WO2[:, DM : 2 * DM]]

            # ---- one-time loads ----
            nc.sync.dma_start(out=WQ, in_=wq.rearrange("(c p) m -> p c m", p=128))
            nc.sync.dma_start(out=WK, in_=wk.rearrange("(c p) m -> p c m", p=128))
            nc.sync.dma_start(out=WV, in_=wv.rearrange("(c p) m -> p c m", p=128))
            nc.sync.dma_start(out=WO[0], in_=wo[0:64, :])
            nc.sync.dma_start(out=WO[1], in_=wo[64:128, :])
            # aug rows: h0 at partitions 64:128, h1 at partitions 0:64
            nc.sync.dma_start(out=KT[0][64:128, :], in_=kaug[0, :, :])
            nc.sync.dma_start(out=KT[1][0:64, :], in_=kaug[1, :, :])
            nc.sync.dma_start(out=QT[0][64:128, :], in_=qaug[0, :, :])
            nc.sync.dma_start(out=QT[1][0:64, :], in_=qaug[1, :, :])
            for h in range(2):
                nc.vector.memset(VA[h], 1.0)

            def _rep_body():
                # ---- projections (head-packed: M=128 covers both heads) ----
                XK, XQ = [], []
                for c in range(N_KC):
                    xk = stage.tile([128, T], bf16, name="stagek", tag="stage")
                    nc.sync.dma_start(out=xk, in_=kT[128 * c : 128 * (c + 1), :])
                    XK.append(xk)
                for c in range(N_KC):
                    xq = stage.tile([128, T], bf16, name="stageq", tag="stage")
                    nc.scalar.dma_start(out=xq, in_=qT[128 * c : 128 * (c + 1), :])
                    XQ.append(xq)
                XV = []
                for c in range(N_KC):
                    xt = stage.tile([128, T], bf16, name="stagev", tag="stage")
                    nc.gpsimd.dma_start(out=xt, in_=vT[128 * c : 128 * (c + 1), :])
                    XV.append(xt)
                # dim rows destination: h0 -> rows 0:64, h1 -> rows 64:128
                for which, W, dst, X, scl in (
                    ("k", WK, KT, XK, 1.0),
                    ("q", WQ, QT, XQ, SCALE),
                ):
                    for tg in range(2):
                        psq = [
                            spsum.tile([128, 512], f32, name="psq", tag="ps")
                            if tt < 2
                            else opsum.tile([128, 512], f32, name="psq2", tag="po")
                            for tt in range(4)
                        ]
                        for c in range(N_KC):
                            for tt in range(4):
                                t = 4 * tg + tt
                                nc.tensor.matmul(
                                    psq[tt],
                                    W[:, c, :],
                                    X[c][:, 512 * t : 512 * (t + 1)],
                                    start=(c == 0),
                                    stop=(c == N_KC - 1),
                                )
                        for tt in range(4):
                            t = 4 * tg + tt
                            for h in range(2):
                                dsl = dst[h][64 * h : 64 * (h + 1), 512 * t : 512 * (t + 1)]
                                src = psq[tt][64 * h : 64 * (h + 1), :]
                                if which == "q":
                                    if tt % 2 == 0:
                                        nc.vector.tensor_scalar_mul(dsl, src, scl)
                                    else:
                                        nc.scalar.mul(dsl, src, scl)
                                else:
                                    if tt % 2 == 0:
                                        nc.vector.tensor_copy(dsl, src)
                                    else:
                                        nc.scalar.copy(dsl, src)
                # V proj: VA chunk t is produced ahead of PV's chunk-j use.
                for t in range(N_JC):
                    ps = opsum.tile([128, 128], f32, name="psv", tag="po")
                    for c in range(N_KC):
                        nc.tensor.matmul(
                            ps,
                            XV[c][:, 128 * t : 128 * (t + 1)],
                            WV[:, c, :],
                            start=(c == 0),
                            stop=(c == N_KC - 1),
                        )
                    for h in range(2):
                        nc.vector.tensor_copy(
                            VA[h][:, 65 * t : 65 * t + 64],
                            ps[:, 64 * h : 64 * (h + 1)],
                        )

                # ---- attention (ib outer, heads inner; per-block epilogue) ----
                # Pipeline: PE issues S_{jj} then PV_{jj-LA}; exp (Act) is the
                # only per-element op between them and paces the loop.
                if "attn" in PHASES:
                    LA = 2
                    pending = []

                    def emit_oproj(ib0, pvs2):
                        for k in range(8):
                            ic = 8 * ib0 + k
                            po = opsum.tile([128, DM], f32, name="po", tag="po")
                            nc.tensor.matmul(
                                po,
                                pvs2[0][0:64, 128 * k : 128 * (k + 1)],
                                WO[0][:, :],
                                start=True,
                                stop=False,
                            )
                            nc.tensor.matmul(
                                po,
                                pvs2[1][0:64, 128 * k : 128 * (k + 1)],
                                WO[1][:, :],
                                start=False,
                                stop=True,
                            )
                            ot = outs.tile([128, DM], f32, name="osb", tag="osb")
                            nc.vector.tensor_copy(ot, po)
                            nc.sync.dma_start(
                                out=out[128 * ic : 128 * (ic + 1), :], in_=ot
                            )

                    for ib in range(N_IB):
                        pvs_cur = []
                        for h in range(2):
                            pv = vpsum.tile([65, IB], f32, name="pv", tag="pv")
                            ess = [None] * N_JC
                            for jj in range(N_JC + LA):
                                if jj < N_JC:
                                    ps = spsum.tile([128, IB], f32, name="ps", tag="ps")
                                    for half in range(IB // 512):
                                        nc.tensor.matmul(
                                            ps[:, 512 * half : 512 * (half + 1)],
                                            KT[h][:, JC * jj : JC * (jj + 1)],
                                            QT[h][
                                                :,
                                                IB * ib + 512 * half : IB * ib + 512 * (half + 1),
                                            ],
                                            start=True,
                                            stop=True,
                                        )
                                    es = esp.tile([128, IB], bf16, name="es", tag="es")
                                    nc.scalar.activation(
                                        es, ps, mybir.ActivationFunctionType.Exp
                                    )
                                    ess[jj] = es
                                if jj >= LA:
                                    j = jj - LA
                                    for half in range(IB // 512):
                                        nc.tensor.matmul(
                                            pv[:, 512 * half : 512 * (half + 1)],
                                            VA[h][:, 65 * j : 65 * (j + 1)],
                                            ess[j][:, 512 * half : 512 * (half + 1)],
                                            start=(j == 0),
                                            stop=(j == N_JC - 1),
                                        )
                                    ess[j] = None
                            # epilogue: drain, 1/l in place, hop the row to
                            # partition 0 (Act-queue DMA -- gpsimd's
                            # partition_broadcast reads via core 0 which only
                            # reaches partitions 0-15), broadcast, normalize.
                            pvt = pvsp.tile([65, IB], bf16, name="pvt", tag="pvt")
                            nc.vector.tensor_copy(pvt, pv[:, :])
                            with nc.allow_low_precision(reason="1/l bf16 ok"):
                                nc.vector.reciprocal(
                                    pvt[64:65, :], pvt[64:65, :]
                                )
                            lt = rbp.tile([1, IB], bf16, name="lt", tag="lt")
                            if FAKE_LT:
                                nc.gpsimd.memset(lt, 0.01)
                            else:
                                nc.gpsimd.dma_start(out=lt, in_=pvt[64:65, :])
                            rb = rbp.tile([64, IB], bf16, name="rb", tag="rb")
                            nc.gpsimd.partition_broadcast(rb, lt[0:1, :])
                            nc.gpsimd.tensor_mul(
                                pvt[0:64, :], pvt[0:64, :], rb
                            )
                            pvs_cur.append(pvt)

                        # output projection, deferred one i-block so the PE
                        # stream never stalls on the normalize chain
                        if "oproj" in PHASES:
                            pending.append((ib, pvs_cur))
                            if len(pending) > 1:
                                emit_oproj(*pending.pop(0))
                    if "oproj" in PHASES:
                        while pending:
                            emit_oproj(*pending.pop(0))

            if loop_reps is None:
                for _rep in range(REPS):
                    _rep_body()
            else:
                with tc.For_i(0, loop_reps, 1):
                    _rep_body()

    nc.compile()
    return nc


# cache of bias-kernel SVD factors keyed by (an, ap) rounded
_AUG_CACHE = {}


def _bias_factors(an, ap):
    """Rank-RANK factorization of g(x,y)=an*(x-y)+c*relu(x-y) on [0,1)^2.

    Returns (tab_j [G, RANK], tab_i [G, RANK]) grid tables such that
    tab_j(x) . tab_i(y) ~= g(x, y); evaluate by linear interpolation.
    """
    key = (round(float(an), 9), round(float(ap), 9))
    if key in _AUG_CACHE:
        return _AUG_CACHE[key]
    G = AUG_GRID
    xs = (np.arange(G) + 0.5) / G
    c = ap - an
    D = xs[:, None] - xs[None, :]
    M = an * D + c * np.maximum(D, 0.0)
    U, s, Vt = np.linalg.svd(M)
    r = RANK
    sq = np.sqrt(s[:r])
    tab_j = U[:, :r] * sq          # K side (f_j)
    tab_i = Vt[:r, :].T * sq       # Q side (f_i)
    _AUG_CACHE[key] = (tab_j, tab_i)
    return _AUG_CACHE[key]


def _interp(tab, f):
    """Linear interpolation of grid table rows at points f in [0,1)."""
    G = tab.shape[0]
    idx = np.clip(f * G - 0.5, 0.0, G - 1.0)
    lo = np.floor(idx).astype(np.int64)
    hi = np.minimum(lo + 1, G - 1)
    w = (idx - lo)[:, None]
    return tab[lo] * (1.0 - w) + tab[hi] * w


def _prep_core(c, query, key, value, frac, Wq, bq, Wk, bk, Wv, bv, Wo,
               alpha_pos, alpha_neg):
    b = c // 4
    hp = c % 4
    h0 = 2 * hp
    sl = slice(64 * h0, 64 * h0 + 128)
    f = frac[b].astype(np.float64)

    def b16(x):
        return np.ascontiguousarray(x).astype(BF16)

    m = {
        "qT": b16(query[b].T),
        "kT": b16(key[b].T),
        "vT": b16(value[b].T),
        "wq": b16(Wq[sl].T),
        "wk": b16(Wk[sl].T),
        "wv": b16(Wv[sl].T),
        "wo": b16(Wo[:, sl].T),
    }
    assert np.all(bq == 0) and np.all(bk == 0) and np.all(bv == 0), (
        "nonzero qkv biases not supported by this kernel"
    )
    ka, qa = [], []
    for h in (h0, h0 + 1):
        an = float(alpha_neg[h])
        ap = float(alpha_pos[h])
        tab_j, tab_i = _bias_factors(an, ap)
        ka.append(_interp(tab_j, f).T)  # [RANK, T]
        qa.append(_interp(tab_i, f).T)  # [RANK, T]
    m["kaug"] = b16(np.stack(ka))
    m["qaug"] = b16(np.stack(qa))
    return m


def kernel(**inputs):
    global _PROGRAM
    inp = {k: np.asarray(v) for k, v in inputs.items()}
    if _PROGRAM is None:
        _PROGRAM = _build_program()

    in_maps = [
        _prep_core(
            c,
            inp["query"], inp["key"], inp["value"], inp["frac"],
            inp["Wq"], inp["bq"], inp["Wk"], inp["bk"],
            inp["Wv"], inp["bv"], inp["Wo"],
            inp["alpha_pos"], inp["alpha_neg"],
        )
        for c in range(N_CORES)
    ]
    res = run_bass_kernel_spmd(_PROGRAM, in_maps, core_ids=list(range(N_CORES)))
    B = inp["query"].shape[0]
    outf = np.zeros((B, T, DM), np.float32)
    for c in range(N_CORES):
        outf[c // 4] += res.results[c]["out"]
    outf += inp["bo"].astype(np.float32)
    return outf.astype(np.float32)


if __name__ == "__main__":
    sys.path.insert(0, "/root/problem")
    import reference

    ins = {k: np.asarray(v) for k, v in reference.setup_inputs().items()}
    got = kernel(**ins)
    exp = np.asarray(reference.reference(**ins))
    err = np.linalg.norm(got - exp) / np.linalg.norm(exp)
    print("rel l2 err:", err)
    print("max abs err:", np.abs(got - exp).max())
